# revision 26
# baseline (speedup 1.0000x reference)
"""Expert-parallel MoE FFN kernel for Trainium2 (8 NeuronCores).

Strategy (sharding_hint: expert-parallel):
  - Host computes the gate in fp32 (softmax -> top-2 -> renormalize) and
    dispatches tokens to experts (the "all-to-all" happens in host staging,
    which is legal because kernel() receives FULL inputs).
  - Core e holds expert e's weights (bf16) and processes its gathered tokens
    (padded to a static capacity C) through the FFN:
        Y = (gelu(X @ W1 + b1) @ W2) * combine_scale
    All GEMMs run in bf16 on the PE with fp32 PSUM accumulation; gelu (exact,
    erf-based) is fused into the PSUM eviction on the scalar engine; the
    combine-weight scaling is fused into the second GEMM's PSUM eviction on
    the vector engine.
  - Host scatters per-expert outputs back (indices are unique per expert) and
    adds the (gate-weighted) b2 term.

Layouts (per core):
  xt  [H, C]  bf16   gathered tokens, transposed (contraction dim on partitions)
  w1  [H, FF] bf16   natural layout == lhsT for GEMM1
  w2  [FF, H] bf16   natural layout == lhsT for GEMM2 (stationary)
  b1p [128, FF/128]  f32, column ff = b1[ff*128:(ff+1)*128]
  y   [H, C]  f32    transposed per-slot FFN output (unscaled)

GEMM1 produces Hmid^T (FF on partitions); GEMM2 keeps tokens on the moving
operand (cycles scale with the exact token count, not 128-padded tiles) and
produces Y^T. The combine-weight scale and the final transpose happen on the
host during the scatter — zero transposes or gather/scatter on device.
"""

import sys

if "/opt/trn_rl_repo" not in sys.path:
    sys.path.insert(0, "/opt/trn_rl_repo")

import numpy as np
import ml_dtypes

H = 1024          # hidden size
E = 8             # experts == cores
TOPK = 2
FF = 4 * H        # expert hidden dim
P = 128           # SBUF partitions
NB = 384          # token block (matmul free dim, <= 512 PSUM bank)
NH = 512          # GEMM2 output free-dim tile

_prog_cache: dict[int, object] = {}
LAST_RESULTS = None  # BassKernelResults of the most recent run (for test harness)
TRACE = False        # test harness can set kernel.TRACE = True for profiling
ACT_OVERRIDE = None  # sim-only: CoreSim lacks Gelu; tests may set e.g. "Relu"
LAST_CALL = None     # (nc, in_maps) of the most recent run, for re-runs


def _build_program(C: int, A: int | None = None):
    """Build + compile the per-core SPMD Bass program.

    C: padded capacity (multiple of 128) — the DRAM/SBUF array width.
    A: active slot count (<= C) — compute covers only slots [0, A);
       y rows [A, C) stay at the runtime's zero-fill.
    """
    from contextlib import ExitStack

    from concourse import bacc
    import concourse.mybir as mybir
    import concourse.tile as tile

    if A is None:
        A = C
    dt = mybir.dt
    KH = H // P            # 8  contraction chunks for GEMM1
    KF = FF // P           # 32 contraction chunks for GEMM2
    assert C % P == 0 and 0 < A <= C
    # token blocks over the active range: full NB blocks plus a remainder
    blocks = []
    t = 0
    while A - t >= NB:
        blocks.append((t, NB))
        t += NB
    if t < A:
        blocks.append((t, A - t))

    nc = bacc.Bacc(None, target_bir_lowering=False, debug=False)

    xt = nc.dram_tensor("xt", [H, C], dt.bfloat16, kind="ExternalInput")
    w1 = nc.dram_tensor("w1", [H, FF], dt.bfloat16, kind="ExternalInput")
    w2 = nc.dram_tensor("w2", [FF, H], dt.bfloat16, kind="ExternalInput")
    b1p = nc.dram_tensor("b1p", [P, KF], dt.float32, kind="ExternalInput")
    y = nc.dram_tensor("y", [H, C], dt.float32, kind="ExternalOutput")

    with ExitStack() as ctx:
        tc = ctx.enter_context(tile.TileContext(nc))
        wpool = ctx.enter_context(tc.tile_pool(name="wpool", bufs=1))
        hpool = ctx.enter_context(tc.tile_pool(name="hpool", bufs=1))
        psA = ctx.enter_context(tc.tile_pool(name="psA", bufs=3, space="PSUM"))
        psB = ctx.enter_context(tc.tile_pool(name="psB", bufs=3, space="PSUM"))
        opool = ctx.enter_context(tc.tile_pool(name="opool", bufs=4))

        # --- resident inputs ------------------------------------------------
        # Few large multi-chunk DMAs: descriptor ISSUE on the sync engine
        # (~0.7us per dma_start) is the startup bottleneck, while one big
        # DMA fans out across all 16 SDMA engines at full fabric BW.
        # Emission order = consumption order: biases, block-0 tokens,
        # W1 ff-blocks (GEMM1 eats one ff-block per ~10us), W2 (needed at
        # ~50% mark), remaining token blocks (needed at ~35%... later).
        xtile = wpool.tile([P, KH, C], dt.bfloat16, tag="xtile", name="xtile")
        w1t = wpool.tile([P, KH, FF], dt.bfloat16, tag="w1t", name="w1t")
        w2t = wpool.tile([P, KF, H], dt.bfloat16, tag="w2t", name="w2t")
        xt_r = xt[:, :].rearrange("(k p) t -> p k t", p=P)
        w1_r = w1[:, :].rearrange("(k p) f -> p k f", p=P)
        w2_r = w2[:, :].rearrange("(k p) h -> p k h", p=P)

        nb0 = blocks[0][1]
        nc.sync.dma_start(out=xtile[:, :, 0:nb0], in_=xt_r[:, :, 0:nb0])
        b1t = wpool.tile([P, KF], dt.float32, tag="b1t", name="b1t")
        nc.sync.dma_start(out=b1t[:], in_=b1p[:])
        # W1 pieces sized so the first matmul group's critical DMA prefix
        # is just xt block 0 + one ff-tile of W1 (~1MB); later pieces
        # coarsen as GEMM1 consumption (one ff-tile per ~1.3us) lags DMA
        w1_edges = [0, P, 4 * P, FF // 4, FF // 2, 3 * FF // 4, FF]
        for fb in range(len(w1_edges) - 1):
            nc.sync.dma_start(
                out=w1t[:, :, w1_edges[fb]:w1_edges[fb + 1]],
                in_=w1_r[:, :, w1_edges[fb]:w1_edges[fb + 1]],
            )
        nc.sync.dma_start(out=w2t[:, :, :], in_=w2_r[:, :, :])
        if A > nb0:
            nc.sync.dma_start(out=xtile[:, :, nb0:A], in_=xt_r[:, :, nb0:A])

        # --- main loop over token blocks ------------------------------------
        act = getattr(mybir.ActivationFunctionType, ACT_OVERRIDE or "Gelu")
        for t0, nb in blocks:
            # GEMM1: HmidT[ff, t] = gelu(sum_h W1[h, ff] * xt[h, t] + b1[ff])
            hblk = hpool.tile([P, KF, NB], dt.bfloat16, tag="hblk", name="hblk")
            for ff in range(KF):
                pa = psA.tile([P, NB], dt.float32, tag="pa", name="pa")
                for k in range(KH):
                    nc.tensor.matmul(
                        pa[:, :nb],
                        lhsT=w1t[:, k, ff * P:(ff + 1) * P],
                        rhs=xtile[:, k, t0:t0 + nb],
                        start=(k == 0),
                        stop=(k == KH - 1),
                    )
                nc.scalar.activation(
                    hblk[:, ff, :nb],
                    pa[:, :nb],
                    act,
                    bias=b1t[:, ff:ff + 1],
                )
            # GEMM2: YT[h, t] = sum_f W2[f, h] * HmidT[f, t]
            # W2 chunks are the stationary operand; tokens stay on the
            # moving side so cycles scale with the exact token count.
            for ht in range(H // P):
                pb = psB.tile([P, NB], dt.float32, tag="pb", name="pb")
                for k in range(KF):
                    nc.tensor.matmul(
                        pb[:, :nb],
                        lhsT=w2t[:, k, ht * P:(ht + 1) * P],
                        rhs=hblk[:, k, :nb],
                        start=(k == 0),
                        stop=(k == KF - 1),
                    )
                ot = opool.tile([P, NB], dt.float32, tag="ot", name="ot")
                nc.vector.tensor_copy(ot[:, :nb], pb[:, :nb])
                nc.sync.dma_start(
                    out=y[ht * P:(ht + 1) * P, t0:t0 + nb], in_=ot[:, :nb]
                )

    nc.compile()
    return nc


def _get_program(C: int, A: int | None = None):
    key = (C, A)
    if key not in _prog_cache:
        _prog_cache[key] = _build_program(C, A)
    return _prog_cache[key]


def _route(xf: np.ndarray, Wg: np.ndarray, bg: np.ndarray):
    """fp32 gate: softmax -> top-2 (stable order, matches jax top_k) -> renorm."""
    logits = xf @ np.asarray(Wg, np.float32) + np.asarray(bg, np.float32)
    m = logits.max(axis=1, keepdims=True)
    p = np.exp(logits - m, dtype=np.float32)
    p /= p.sum(axis=1, keepdims=True)
    order = np.argsort(-p, axis=1, kind="stable")
    idx = order[:, :TOPK]
    pv = np.take_along_axis(p, idx, axis=1)
    vals = (pv / pv.sum(axis=1, keepdims=True)).astype(np.float32)
    return idx, vals


def kernel(x, Wg, bg, W1, b1, W2, b2):
    global LAST_RESULTS
    from concourse.bass_utils import run_bass_kernel_spmd

    x = np.asarray(x, np.float32)
    xf = x.reshape(-1, H)
    T = xf.shape[0]

    idx, vals = _route(xf, Wg, bg)

    counts = np.bincount(idx.ravel(), minlength=E)
    A = int(counts.max())
    C = max(P, -(-A // P) * P)

    nc = _get_program(C, A)

    bf16 = ml_dtypes.bfloat16
    W1 = np.asarray(W1, np.float32)
    W2 = np.asarray(W2, np.float32)
    b1 = np.asarray(b1, np.float32)
    KF = FF // P

    in_maps = []
    ids_list = []
    for e in range(E):
        sel = idx == e                      # [T, 2]; at most one True per row
        ids = np.nonzero(sel.any(axis=1))[0]
        sc = vals[sel]                      # row-major => aligned with ids
        cnt = ids.shape[0]

        xe = np.zeros((C, H), np.float32)
        xe[:cnt] = xf[ids]

        in_maps.append({
            "xt": np.ascontiguousarray(xe.T).astype(bf16),
            "w1": W1[e].astype(bf16),
            "w2": W2[e].astype(bf16),
            "b1p": np.ascontiguousarray(b1[e].reshape(KF, P).T),
        })
        ids_list.append((ids, sc))

    global LAST_CALL
    LAST_CALL = (nc, in_maps)
    LAST_RESULTS = run_bass_kernel_spmd(nc, in_maps, list(range(E)), trace=TRACE)

    out = np.zeros((T, H), np.float32)
    for e in range(E):
        ids, sc = ids_list[e]
        yt = LAST_RESULTS.results[e]["y"]          # [H, C], unscaled
        out[ids] += yt[:, : ids.shape[0]].T * sc[:, None]

    b2 = np.asarray(b2, np.float32)
    out += vals[:, 0:1] * b2[idx[:, 0]] + vals[:, 1:2] * b2[idx[:, 1]]
    return out.reshape(x.shape)
